# revision 20
# baseline (speedup 1.0000x reference)
"""nn_PhaseAwareAttention kernel for 8 Trainium2 NeuronCores.

Algebraic collapse: softmax over a size-1 axis is identically 1, so the
q/k branch (and both node gathers) never affect the output:

    out = edge_attr + 0.5*(((edge_attr @ Wv.T + bv) @ Wiv.T + biv) @ W_mo.T
                           + b_mo) @ Wo.T + bo
        = edge_attr @ (I + 0.5*(Wo @ W_mo @ Wiv @ Wv).T) + c

The identity is folded into the 128x128 weight so the device kernel is a
pure streamed matmul: yT = (I+M).T @ xT per 512-edge column block, with
edge_attr sharded over 8 cores and transposed to [HID, E/8] so the
contraction dim sits on partitions.

Bacc's legalization pass (generate_event_semaphores) splits multi-wait
instructions, so the matmul can read the DMA-landed x tile directly.
PSUM drains through DVE copies (PSUM is not a DMA-legal space); x-in
DMAs issue on the SP ring and y-out on the ACT ring so the two HWDGE
queues stream independently.
"""

import numpy as np

import concourse.bacc as bacc
import concourse.bass as bass
import concourse.mybir as mybir
from concourse.bass_utils import run_bass_kernel_spmd
from concourse.tile import TileContext

E = 250000
HID = 128
NCORES = 8
ESH = E // NCORES          # 31250 edges per core
BIG = 4096                 # edges per DMA chunk
SUB = 512                  # edges per matmul (one PSUM bank of fp32)
# Small first chunk -> first matmul/drain starts early; tapered last
# chunks -> short drain+store tail after the x stream ends.
CHUNKS = [1024] + [4096] * 7 + [1024, 530]
assert sum(CHUNKS) == ESH

_PROGRAM_CACHE = {}


def _build_program():
    if "nc" in _PROGRAM_CACHE:
        return _PROGRAM_CACHE["nc"]

    nc = bacc.Bacc()
    f32 = mybir.dt.float32
    xt = nc.dram_tensor("xt", [HID, ESH], f32, kind="ExternalInput")
    wm = nc.dram_tensor("wm", [HID, HID], f32, kind="ExternalInput")
    yt = nc.dram_tensor("yt", [HID, ESH], f32, kind="ExternalOutput")

    with TileContext(nc) as tc:
        with (
            tc.tile_pool(name="const", bufs=1) as cpool,
            tc.tile_pool(name="xraw", bufs=5) as rpool,
            tc.tile_pool(name="yout", bufs=4) as opool,
            tc.tile_pool(name="psum", bufs=8, space="PSUM") as ppool,
        ):
            w_tile = cpool.tile([HID, HID], f32)
            nc.scalar.dma_start(out=w_tile, in_=wm[:, :])

            c0 = 0
            for cw in CHUNKS:
                x_raw = rpool.tile([HID, BIG], f32)
                nc.sync.dma_start(out=x_raw[:, :cw], in_=xt[:, c0 : c0 + cw])
                o_tile = opool.tile([HID, BIG], f32)
                for s in range(0, cw, SUB):
                    n = min(SUB, cw - s)
                    ps = ppool.tile([HID, SUB], f32)
                    nc.tensor.matmul(
                        ps[:, :n], w_tile, x_raw[:, s : s + n],
                        start=True, stop=True,
                    )
                    nc.vector.tensor_copy(o_tile[:, s : s + n], ps[:, :n])
                nc.scalar.dma_start(out=yt[:, c0 : c0 + cw], in_=o_tile[:, :cw])
                c0 += cw

    nc.finalize()
    _PROGRAM_CACHE["nc"] = nc
    return nc


def _prepare(inputs):
    x = np.ascontiguousarray(inputs["edge_attr"], dtype=np.float32)

    Wv = inputs["Wv"].astype(np.float64)
    bv = inputs["bv"].astype(np.float64)
    W_in = inputs["W_in"].astype(np.float64)
    b_in = inputs["b_in"].astype(np.float64)
    Wiv = W_in[2 * HID : 3 * HID]
    biv = b_in[2 * HID : 3 * HID]
    W_mo = inputs["W_mo"].astype(np.float64)
    b_mo = inputs["b_mo"].astype(np.float64)
    Wo = inputs["Wo"].astype(np.float64)
    bo = inputs["bo"].astype(np.float64)

    M = 0.5 * (Wo @ W_mo @ Wiv @ Wv).T
    c = 0.5 * (((bv @ Wiv.T + biv) @ W_mo.T + b_mo) @ Wo.T + bo)

    wm = np.ascontiguousarray(np.eye(HID) + M, dtype=np.float32)
    cf = c.astype(np.float32)

    nc = _build_program()

    in_maps = []
    for i in range(NCORES):
        shard = x[i * ESH : (i + 1) * ESH]
        in_maps.append({"xt": np.ascontiguousarray(shard.T), "wm": wm})

    return nc, in_maps, cf


def kernel(**inputs) -> np.ndarray:
    nc, in_maps, cf = _prepare(inputs)

    res = run_bass_kernel_spmd(nc, in_maps, list(range(NCORES)))

    out = np.empty((E, HID), dtype=np.float32)
    for i in range(NCORES):
        out[i * ESH : (i + 1) * ESH] = res.results[i]["yt"].T
    if np.any(cf != 0.0):
        out += cf[None, :]
    return out


# revision 21
# speedup vs baseline: 1.1263x; 1.1263x over previous
"""nn_PhaseAwareAttention kernel for 8 Trainium2 NeuronCores.

Algebraic collapse: softmax over a size-1 axis is identically 1, so the
q/k branch (and both node gathers) never affect the output:

    out = edge_attr + 0.5*(((edge_attr @ Wv.T + bv) @ Wiv.T + biv) @ W_mo.T
                           + b_mo) @ Wo.T + bo
        = edge_attr @ (I + 0.5*(Wo @ W_mo @ Wiv @ Wv).T) + c

The identity is folded into the 128x128 weight so the device kernel is a
pure streamed matmul: yT = (I+M).T @ xT per 512-edge column block, with
edge_attr sharded over 8 cores and transposed to [HID, E/8] so the
contraction dim sits on partitions.

Bacc's legalization pass (generate_event_semaphores) splits multi-wait
instructions, so the matmul can read the DMA-landed x tile directly.
PSUM drains through DVE copies (PSUM is not a DMA-legal space); x-in
DMAs issue on the SP ring and y-out on the ACT ring so the two HWDGE
queues stream independently.
"""

import numpy as np

import concourse.bacc as bacc
import concourse.bass as bass
import concourse.mybir as mybir
from concourse.bass_utils import run_bass_kernel_spmd
from concourse.tile import TileContext

E = 250000
HID = 128
NCORES = 8
ESH = E // NCORES          # 31250 edges per core
BIG = 4096                 # edges per DMA chunk
SUB = 512                  # edges per matmul (one PSUM bank of fp32)
# Small first chunk -> first matmul/drain starts early; tapered last
# chunks -> short drain+store tail after the x stream ends.
CHUNKS = [1024] + [4096] * 7 + [1024, 530]
assert sum(CHUNKS) == ESH

_PROGRAM_CACHE = {}


def _build_program():
    if "nc" in _PROGRAM_CACHE:
        return _PROGRAM_CACHE["nc"]

    nc = bacc.Bacc()
    f32 = mybir.dt.float32
    xt = nc.dram_tensor("xt", [HID, ESH], f32, kind="ExternalInput")
    wm = nc.dram_tensor("wm", [HID, HID], f32, kind="ExternalInput")
    yt = nc.dram_tensor("yt", [HID, ESH], f32, kind="ExternalOutput")

    with TileContext(nc) as tc:
        with (
            tc.tile_pool(name="const", bufs=1) as cpool,
            tc.tile_pool(name="xraw", bufs=4) as rpool,
            tc.tile_pool(name="yout", bufs=4) as opool,
            tc.tile_pool(name="psum", bufs=8, space="PSUM") as ppool,
        ):
            w_tile = cpool.tile([HID, HID], f32)
            nc.scalar.dma_start(out=w_tile, in_=wm[:, :])

            c0 = 0
            for cw in CHUNKS:
                x_raw = rpool.tile([HID, BIG], f32)
                nc.sync.dma_start(out=x_raw[:, :cw], in_=xt[:, c0 : c0 + cw])
                o_tile = opool.tile([HID, BIG], f32)
                for s in range(0, cw, SUB):
                    n = min(SUB, cw - s)
                    ps = ppool.tile([HID, SUB], f32)
                    nc.tensor.matmul(
                        ps[:, :n], w_tile, x_raw[:, s : s + n],
                        start=True, stop=True,
                    )
                    nc.vector.tensor_copy(o_tile[:, s : s + n], ps[:, :n])
                nc.scalar.dma_start(out=yt[:, c0 : c0 + cw], in_=o_tile[:, :cw])
                c0 += cw

    nc.finalize()
    _PROGRAM_CACHE["nc"] = nc
    return nc


def _prepare(inputs):
    x = np.ascontiguousarray(inputs["edge_attr"], dtype=np.float32)

    Wv = inputs["Wv"].astype(np.float64)
    bv = inputs["bv"].astype(np.float64)
    W_in = inputs["W_in"].astype(np.float64)
    b_in = inputs["b_in"].astype(np.float64)
    Wiv = W_in[2 * HID : 3 * HID]
    biv = b_in[2 * HID : 3 * HID]
    W_mo = inputs["W_mo"].astype(np.float64)
    b_mo = inputs["b_mo"].astype(np.float64)
    Wo = inputs["Wo"].astype(np.float64)
    bo = inputs["bo"].astype(np.float64)

    M = 0.5 * (Wo @ W_mo @ Wiv @ Wv).T
    c = 0.5 * (((bv @ Wiv.T + biv) @ W_mo.T + b_mo) @ Wo.T + bo)

    wm = np.ascontiguousarray(np.eye(HID) + M, dtype=np.float32)
    cf = c.astype(np.float32)

    nc = _build_program()

    in_maps = []
    for i in range(NCORES):
        shard = x[i * ESH : (i + 1) * ESH]
        in_maps.append({"xt": np.ascontiguousarray(shard.T), "wm": wm})

    return nc, in_maps, cf


def kernel(**inputs) -> np.ndarray:
    nc, in_maps, cf = _prepare(inputs)

    res = run_bass_kernel_spmd(nc, in_maps, list(range(NCORES)))

    out = np.empty((E, HID), dtype=np.float32)
    for i in range(NCORES):
        out[i * ESH : (i + 1) * ESH] = res.results[i]["yt"].T
    if np.any(cf != 0.0):
        out += cf[None, :]
    return out
